# revision 11
# baseline (speedup 1.0000x reference)
"""BOW regression kernel for Trainium2 (8 NeuronCores, data-parallel over batch).

Per NeuronCore (512 batch columns of the 4096):
  - column-on-partition layout: partition p = 16*g + q holds 4 columns
    (slot s in 0..3) of 200 tokens each; column-local id c = s*16 + q of
    Q7-group g; global batch b = nc*512 + g*64 + c.
  - exact set-dedup: per column, iterative 8-wide max-sort
    (vector.max + match_replace) makes duplicates adjacent, then
    adjacent not_equal builds a keep mask; dropped duplicates are
    remapped to token 1 (the pad token, whose table entry is zero).
  - gather: W is pre-chunked into 16 chunks of 8192 (f32, 32KB) with one
    chunk per partition within each Q7 group; gpsimd.indirect_copy
    gathers table[p, x & 8191] for every stream token.  A second
    indirect_copy into a constant mask table T2[p, x >> 4] yields
    (x >> 13 == p % 16), selecting the one partition holding the right
    chunk; both index tensors are already in the 16-wrapped layout the
    instruction expects (stream pos j = f*16 + q).
  - reduce: PE matmul against a 128x8 group-indicator contracts the 16
    partitions of each group while accumulating over token blocks;
    final 8-wide free-dim reduce + sigmoid(+bias) on DVE/ACT.
"""

import os
import sys

import numpy as np

sys.path.insert(0, "/opt/trn_rl_repo")

T = 200
B = 4096
V = 100000
NC_COUNT = 8
NCOL = 512  # batch columns per NeuronCore
CHUNK = 8192  # vocab chunk per partition (uint16-indexable, 32KB f32)
GROUPS = 8  # Q7 groups per NeuronCore
COLS_PER_GROUP = 64
SLOTS = 4  # columns per partition
NIDX = COLS_PER_GROUP * T  # gather stream length per group = 12800
T2N = 6256  # mask table entries (>= ceil(V/16)=6250, mult of 16)

_prog_cache = {}


def _build_program():
    import concourse.bass as bass
    import concourse.mybir as mybir
    import concourse.tile as tile
    from concourse import bacc

    dt = mybir.dt
    Alu = mybir.AluOpType

    nc = bacc.Bacc(
        "TRN2", target_bir_lowering=False, debug=False, num_devices=NC_COUNT
    )

    text_in = nc.dram_tensor("text_cols", [128, SLOTS * T], dt.int32, kind="ExternalInput")
    table_in = nc.dram_tensor("table", [128, CHUNK], dt.float32, kind="ExternalInput")
    mask_in = nc.dram_tensor("masktab", [128, T2N], dt.float32, kind="ExternalInput")
    ind_in = nc.dram_tensor("ind", [128, GROUPS], dt.float32, kind="ExternalInput")
    bias_in = nc.dram_tensor("bias", [GROUPS, 1], dt.float32, kind="ExternalInput")
    out_t = nc.dram_tensor("scores", [GROUPS, COLS_PER_GROUP], dt.float32, kind="ExternalOutput")

    from contextlib import ExitStack

    with ExitStack() as ctx:
        tc = ctx.enter_context(tile.TileContext(nc))
        pool = ctx.enter_context(tc.tile_pool(name="main", bufs=1))
        ppool = ctx.enter_context(tc.tile_pool(name="psum", bufs=1, space="PSUM"))

        # ---- loads -------------------------------------------------------
        x_i32 = pool.tile([128, SLOTS * T], dt.int32, tag="x_i32")
        nc.sync.dma_start(x_i32[:], text_in[:])
        tabl = pool.tile([128, CHUNK], dt.float32, tag="tabl")
        nc.sync.dma_start(tabl[:], table_in[:])
        mtab = pool.tile([128, T2N], dt.float32, tag="mtab")
        nc.sync.dma_start(mtab[:], mask_in[:])
        ind_sb = pool.tile([128, GROUPS], dt.float32, tag="ind_sb")
        nc.sync.dma_start(ind_sb[:], ind_in[:])
        bias_sb = pool.tile([GROUPS, 1], dt.float32, tag="bias_sb")
        nc.sync.dma_start(bias_sb[:], bias_in[:])

        # ---- sort each column (descending) so duplicates are adjacent ----
        work = pool.tile([128, SLOTS * T], dt.float32, tag="work")
        nc.vector.tensor_copy(work[:], x_i32[:])  # exact: tokens < 2^24
        sortd = pool.tile([128, SLOTS * (T + 1)], dt.float32, tag="sortd")
        for s in range(SLOTS):
            base = s * (T + 1)
            nc.vector.memset(sortd[:, base : base + 1], -1.0)  # sentinel
            wslot = work[:, s * T : (s + 1) * T]
            for i in range(T // 8):
                mx = sortd[:, base + 1 + 8 * i : base + 9 + 8 * i]
                nc.vector.max(out=mx, in_=wslot)
                nc.vector.match_replace(
                    out=wslot, in_to_replace=mx, in_values=wslot, imm_value=-3.0e38
                )

        # ---- dedup mask + remap dups/pad to token 1 ----------------------
        keep = pool.tile([128, SLOTS * T], dt.float32, tag="keep")
        xd = pool.tile([128, SLOTS * T], dt.float32, tag="xd")
        for s in range(SLOTS):
            base = s * (T + 1)
            cur = sortd[:, base + 1 : base + 1 + T]
            prv = sortd[:, base : base + T]
            kslot = keep[:, s * T : (s + 1) * T]
            nc.vector.tensor_tensor(out=kslot, in0=cur, in1=prv, op=Alu.not_equal)
            # xd = (sorted - 1) * keep  (then +1 below)
            nc.vector.scalar_tensor_tensor(
                out=xd[:, s * T : (s + 1) * T],
                in0=cur,
                scalar=1.0,
                in1=kslot,
                op0=Alu.subtract,
                op1=Alu.mult,
            )
        nc.vector.tensor_scalar_add(xd[:], xd[:], 1.0)

        # ---- index tensors: o = x & 8191, u = x >> 4 ---------------------
        xi = pool.tile([128, SLOTS * T], dt.int32, tag="xi")
        nc.vector.tensor_copy(xi[:], xd[:])
        oi = pool.tile([128, SLOTS * T], dt.int32, tag="oi")
        nc.vector.tensor_scalar(oi[:], xi[:], CHUNK - 1, None, Alu.bitwise_and)
        o16 = pool.tile([128, SLOTS * T], dt.int16, tag="o16")
        nc.vector.tensor_copy(o16[:], oi[:])
        ui = pool.tile([128, SLOTS * T], dt.int32, tag="ui")
        nc.vector.tensor_scalar(ui[:], xi[:], 4, None, Alu.logical_shift_right)
        u16 = pool.tile([128, SLOTS * T], dt.int16, tag="u16")
        nc.vector.tensor_copy(u16[:], ui[:])

        # ---- gathers (stream pos j = (s*200+t)*16 + q) -------------------
        # HW caps IndirectCopy at 1024 gathered elements per instruction
        GCH = 800  # indices per chunk (50 per partition)
        val = pool.tile([128, NIDX], dt.float32, tag="val")
        msk = pool.tile([128, NIDX], dt.float32, tag="msk")
        for i in range(NIDX // GCH):
            iw = GCH // 16
            nc.gpsimd.ap_gather(
                val[:, i * GCH : (i + 1) * GCH],
                tabl[:],
                o16[:, i * iw : (i + 1) * iw],
                channels=128,
                num_elems=CHUNK,
                d=1,
                num_idxs=GCH,
            )
            nc.gpsimd.ap_gather(
                msk[:, i * GCH : (i + 1) * GCH],
                mtab[:],
                u16[:, i * iw : (i + 1) * iw],
                channels=128,
                num_elems=T2N,
                d=1,
                num_idxs=GCH,
            )

        # ---- chunk select (in place, SBUF is tight) ----------------------
        vsel = val
        nc.vector.tensor_mul(vsel[:], val[:], msk[:])

        # ---- PE reduce: contract partitions within each group ------------
        # stream j = s*3200 + t*16 + q ; accumulate over t blocks of 8
        psum = ppool.tile([GROUPS, 512], dt.float32, tag="psum")
        v4 = vsel[:].rearrange("p (s t q) -> p s t q", s=SLOTS, t=T)
        nblk = T // 8  # 25 accumulation steps
        for r in range(nblk):
            nc.tensor.matmul(
                psum[:],
                ind_sb[:],
                v4[:, :, 8 * r : 8 * (r + 1), :],
                start=(r == 0),
                stop=(r == nblk - 1),
            )

        # ---- final 8-wide reduce + sigmoid -------------------------------
        # psum n = s*128 + ti*16 + q ; reduce ti, output col = s*16 + q
        red = pool.tile([GROUPS, COLS_PER_GROUP], dt.float32, tag="red")
        psum3 = psum[:].rearrange("g (s i q) -> g s q i", s=SLOTS, i=8)
        nc.vector.tensor_reduce(
            out=red[:], in_=psum3, axis=mybir.AxisListType.X, op=Alu.add
        )
        final = pool.tile([GROUPS, COLS_PER_GROUP], dt.float32, tag="final")
        nc.scalar.activation(
            out=final[:],
            in_=red[:],
            func=mybir.ActivationFunctionType.Sigmoid,
            bias=bias_sb[:, 0:1],
            scale=1.0,
        )
        nc.sync.dma_start(out_t[:], final[:])

    nc.finalize()
    return nc


def _get_program():
    if "prog" not in _prog_cache:
        _prog_cache["prog"] = _build_program()
    return _prog_cache["prog"]


def kernel(text, W, b):
    from concourse.bass_utils import run_bass_kernel_spmd

    text = np.asarray(text)
    W = np.asarray(W, dtype=np.float32).reshape(-1)
    b = np.asarray(b, dtype=np.float32).reshape(-1)
    x = text.astype(np.int32)  # [T, B]

    # host-side constant marshalling (input-independent transforms only)
    Wp = np.zeros(16 * CHUNK, np.float32)
    Wp[:V] = W
    Wp[1] = 0.0  # pad token never contributes
    table = np.ascontiguousarray(np.tile(Wp.reshape(16, CHUNK), (GROUPS, 1)))
    masktab = (
        (np.arange(T2N)[None, :] >> 9) == (np.arange(128)[:, None] % 16)
    ).astype(np.float32)
    ind = np.zeros((128, GROUPS), np.float32)
    ind[np.arange(128), np.arange(128) // 16] = 1.0
    bias = np.full((GROUPS, 1), b[0], np.float32)

    in_maps = []
    for d in range(NC_COUNT):
        tb = x[:, d * NCOL : (d + 1) * NCOL]  # [200, 512]
        tbr = tb.reshape(T, GROUPS, SLOTS, 16)  # [t, g, s, q]
        dev = np.ascontiguousarray(tbr.transpose(1, 3, 2, 0).reshape(128, SLOTS * T))
        in_maps.append(
            {
                "text_cols": dev,
                "table": table,
                "masktab": masktab,
                "ind": ind,
                "bias": bias,
            }
        )

    prog = _get_program()
    res = run_bass_kernel_spmd(prog, in_maps, core_ids=list(range(NC_COUNT)))

    out = np.empty((B,), np.float32)
    for d in range(NC_COUNT):
        out[d * NCOL : (d + 1) * NCOL] = res.results[d]["scores"].reshape(NCOL)
    return out.reshape(B, 1)


def benchmark(text, W, b, iters=20):
    """Estimate device execution time: device-resident inputs, repeated
    dispatch of the compiled 8-core program, min wall time per iteration."""
    import time

    import jax
    import numpy as np
    from jax.sharding import Mesh, PartitionSpec
    from jax.experimental.shard_map import shard_map
    from concourse import bass2jax
    import concourse.mybir as mybir

    prog = _get_program()
    # reuse kernel() marshalling
    text = np.asarray(text)
    W = np.asarray(W, dtype=np.float32).reshape(-1)
    b = np.asarray(b, dtype=np.float32).reshape(-1)
    x = text.astype(np.int32)
    Wp = np.zeros(16 * CHUNK, np.float32)
    Wp[:V] = W
    Wp[1] = 0.0
    table = np.ascontiguousarray(np.tile(Wp.reshape(16, CHUNK), (GROUPS, 1)))
    masktab = (
        (np.arange(T2N)[None, :] >> 9) == (np.arange(128)[:, None] % 16)
    ).astype(np.float32)
    ind = np.zeros((128, GROUPS), np.float32)
    ind[np.arange(128), np.arange(128) // 16] = 1.0
    bias = np.full((GROUPS, 1), b[0], np.float32)
    in_maps = []
    for d in range(NC_COUNT):
        tb = x[:, d * NCOL : (d + 1) * NCOL]
        tbr = tb.reshape(T, GROUPS, SLOTS, 16)
        dev = np.ascontiguousarray(tbr.transpose(1, 3, 2, 0).reshape(128, SLOTS * T))
        in_maps.append(
            {"text_cols": dev, "table": table, "masktab": masktab, "ind": ind, "bias": bias}
        )

    bass2jax.install_neuronx_cc_hook()
    nc = prog
    partition_name = nc.partition_id_tensor.name if nc.partition_id_tensor else None
    in_names, out_names, out_avals, zero_outs = [], [], [], []
    for alloc in nc.m.functions[0].allocations:
        if not isinstance(alloc, mybir.MemoryLocationSet):
            continue
        name = alloc.memorylocations[0].name
        if alloc.kind == "ExternalInput":
            if name != partition_name:
                in_names.append(name)
        elif alloc.kind == "ExternalOutput":
            out_names.append(name)
            shape = tuple(alloc.tensor_shape)
            dtype = mybir.dt.np(alloc.dtype)
            out_avals.append(jax.core.ShapedArray(shape, dtype))
            zero_outs.append(np.zeros(shape, dtype))
    n_params = len(in_names)
    n_outs = len(out_avals)
    all_names = in_names + out_names
    if partition_name is not None:
        all_names = all_names + [partition_name]

    def _body(*args):
        operands = list(args)
        if partition_name is not None:
            operands.append(bass2jax.partition_id_tensor())
        outs = bass2jax._bass_exec_p.bind(
            *operands,
            out_avals=tuple(out_avals),
            in_names=tuple(all_names),
            out_names=tuple(out_names),
            lowering_input_output_aliases=(),
            sim_require_finite=True,
            sim_require_nnan=True,
            nc=nc,
        )
        return tuple(outs)

    devices = jax.devices()[:NC_COUNT]
    mesh = Mesh(np.asarray(devices), ("core",))
    in_specs = (PartitionSpec("core"),) * (n_params + n_outs)
    out_specs = (PartitionSpec("core"),) * n_outs
    donate = tuple(range(n_params, n_params + n_outs))
    fn = jax.jit(
        shard_map(_body, mesh=mesh, in_specs=in_specs, out_specs=out_specs, check_rep=False),
        donate_argnums=donate,
        keep_unused=True,
    )
    concat_in = [
        np.concatenate([np.asarray(in_maps[c][nm]) for c in range(NC_COUNT)], axis=0)
        for nm in in_names
    ]
    sh = jax.sharding.NamedSharding(mesh, PartitionSpec("core"))
    dev_in = [jax.device_put(a, sh) for a in concat_in]

    def one_iter():
        zs = [np.zeros((NC_COUNT * z.shape[0], *z.shape[1:]), z.dtype) for z in zero_outs]
        outs = fn(*dev_in, *zs)
        jax.block_until_ready(outs)
        return outs

    one_iter()  # warmup / compile
    times = []
    for _ in range(iters):
        t0 = time.perf_counter()
        one_iter()
        times.append(time.perf_counter() - t0)
    tmin = min(times)
    tmed = sorted(times)[len(times) // 2]
    return tmin, tmed


# revision 12
# speedup vs baseline: 1.2031x; 1.2031x over previous
"""BOW regression kernel for Trainium2 (8 NeuronCores, data-parallel over batch).

Per NeuronCore (512 batch columns of the 4096):
  - column-on-partition layout: partition p = 16*g + q holds 4 columns
    (slot s in 0..3) of 200 tokens each; column-local id c = s*16 + q of
    Q7-group g; global batch b = nc*512 + g*64 + c.
  - exact set-dedup: per column, iterative 8-wide max-sort
    (vector.max + match_replace) makes duplicates adjacent, then
    adjacent not_equal builds a keep mask; dropped duplicates are
    remapped to token 1 (the pad token, whose table entry is zero).
  - gather: W is pre-chunked into 16 chunks of 8192 (f32, 32KB) with one
    chunk per partition within each Q7 group; gpsimd.indirect_copy
    gathers table[p, x & 8191] for every stream token.  A second
    indirect_copy into a constant mask table T2[p, x >> 4] yields
    (x >> 13 == p % 16), selecting the one partition holding the right
    chunk; both index tensors are already in the 16-wrapped layout the
    instruction expects (stream pos j = f*16 + q).
  - reduce: PE matmul against a 128x8 group-indicator contracts the 16
    partitions of each group while accumulating over token blocks;
    final 8-wide free-dim reduce + sigmoid(+bias) on DVE/ACT.
"""

import os
import sys

import numpy as np

sys.path.insert(0, "/opt/trn_rl_repo")

T = 200
B = 4096
V = 100000
NC_COUNT = 8
NCOL = 512  # batch columns per NeuronCore
CHUNK = 8192  # vocab chunk per partition (uint16-indexable, 32KB f32)
GROUPS = 8  # Q7 groups per NeuronCore
COLS_PER_GROUP = 64
SLOTS = 4  # columns per partition
NIDX = COLS_PER_GROUP * T  # gather stream length per group = 12800
T2N = 6256  # mask table entries (>= ceil(V/16)=6250, mult of 16)

_prog_cache = {}


def _build_program():
    import concourse.bass as bass
    import concourse.mybir as mybir
    import concourse.tile as tile
    from concourse import bacc

    dt = mybir.dt
    Alu = mybir.AluOpType

    nc = bacc.Bacc(
        "TRN2", target_bir_lowering=False, debug=False, num_devices=NC_COUNT
    )

    text_in = nc.dram_tensor("text_cols", [128, SLOTS * T], dt.int32, kind="ExternalInput")
    table_in = nc.dram_tensor("table", [128, CHUNK], dt.float32, kind="ExternalInput")
    mask_in = nc.dram_tensor("masktab", [128, T2N], dt.float32, kind="ExternalInput")
    ind_in = nc.dram_tensor("ind", [128, GROUPS], dt.float32, kind="ExternalInput")
    bias_in = nc.dram_tensor("bias", [GROUPS, 1], dt.float32, kind="ExternalInput")
    out_t = nc.dram_tensor("scores", [GROUPS, COLS_PER_GROUP], dt.float32, kind="ExternalOutput")

    from contextlib import ExitStack

    with ExitStack() as ctx:
        tc = ctx.enter_context(tile.TileContext(nc))
        pool = ctx.enter_context(tc.tile_pool(name="main", bufs=1))
        ppool = ctx.enter_context(tc.tile_pool(name="psum", bufs=1, space="PSUM"))

        # ---- loads -------------------------------------------------------
        x_i32 = pool.tile([128, SLOTS * T], dt.int32, tag="x_i32")
        nc.sync.dma_start(x_i32[:], text_in[:])
        tabl = pool.tile([128, CHUNK], dt.float32, tag="tabl")
        nc.sync.dma_start(tabl[:], table_in[:])
        mtab = pool.tile([128, T2N], dt.float32, tag="mtab")
        nc.sync.dma_start(mtab[:], mask_in[:])
        ind_sb = pool.tile([128, GROUPS], dt.float32, tag="ind_sb")
        nc.sync.dma_start(ind_sb[:], ind_in[:])
        bias_sb = pool.tile([GROUPS, 1], dt.float32, tag="bias_sb")
        nc.sync.dma_start(bias_sb[:], bias_in[:])

        # ---- sort each column (descending) so duplicates are adjacent ----
        work = pool.tile([128, SLOTS * T], dt.float32, tag="work")
        nc.vector.tensor_copy(work[:], x_i32[:])  # exact: tokens < 2^24
        sortd = pool.tile([128, SLOTS * (T + 1)], dt.float32, tag="sortd")
        for s in range(SLOTS):
            base = s * (T + 1)
            nc.vector.memset(sortd[:, base : base + 1], -1.0)  # sentinel
            wslot = work[:, s * T : (s + 1) * T]
            for i in range(T // 8):
                mx = sortd[:, base + 1 + 8 * i : base + 9 + 8 * i]
                nc.vector.max(out=mx, in_=wslot)
                nc.vector.match_replace(
                    out=wslot, in_to_replace=mx, in_values=wslot, imm_value=-3.0e38
                )

        # ---- dedup mask + remap dups/pad to token 1 ----------------------
        keep = pool.tile([128, SLOTS * T], dt.float32, tag="keep")
        xd = pool.tile([128, SLOTS * T], dt.float32, tag="xd")
        for s in range(SLOTS):
            base = s * (T + 1)
            cur = sortd[:, base + 1 : base + 1 + T]
            prv = sortd[:, base : base + T]
            kslot = keep[:, s * T : (s + 1) * T]
            nc.vector.tensor_tensor(out=kslot, in0=cur, in1=prv, op=Alu.not_equal)
            # xd = (sorted - 1) * keep  (then +1 below)
            nc.vector.scalar_tensor_tensor(
                out=xd[:, s * T : (s + 1) * T],
                in0=cur,
                scalar=1.0,
                in1=kslot,
                op0=Alu.subtract,
                op1=Alu.mult,
            )
        nc.vector.tensor_scalar_add(xd[:], xd[:], 1.0)

        # ---- index tensors: o = x & 8191, u = x >> 4 ---------------------
        xi = pool.tile([128, SLOTS * T], dt.int32, tag="xi")
        nc.vector.tensor_copy(xi[:], xd[:])
        oi = pool.tile([128, SLOTS * T], dt.int32, tag="oi")
        nc.vector.tensor_scalar(oi[:], xi[:], CHUNK - 1, None, Alu.bitwise_and)
        o16 = pool.tile([128, SLOTS * T], dt.int16, tag="o16")
        nc.vector.tensor_copy(o16[:], oi[:])
        ui = pool.tile([128, SLOTS * T], dt.int32, tag="ui")
        nc.vector.tensor_scalar(ui[:], xi[:], 4, None, Alu.logical_shift_right)
        u16 = pool.tile([128, SLOTS * T], dt.int16, tag="u16")
        nc.vector.tensor_copy(u16[:], ui[:])

        # ---- gathers (stream pos j = (s*200+t)*16 + q) -------------------
        GCH = 3200  # indices per ap_gather instruction (200 per partition)
        val = pool.tile([128, NIDX], dt.float32, tag="val")
        msk = pool.tile([128, NIDX], dt.float32, tag="msk")
        for i in range(NIDX // GCH):
            iw = GCH // 16
            nc.gpsimd.ap_gather(
                val[:, i * GCH : (i + 1) * GCH],
                tabl[:],
                o16[:, i * iw : (i + 1) * iw],
                channels=128,
                num_elems=CHUNK,
                d=1,
                num_idxs=GCH,
            )
            nc.gpsimd.ap_gather(
                msk[:, i * GCH : (i + 1) * GCH],
                mtab[:],
                u16[:, i * iw : (i + 1) * iw],
                channels=128,
                num_elems=T2N,
                d=1,
                num_idxs=GCH,
            )

        # ---- chunk select (in place, SBUF is tight) ----------------------
        vsel = val
        nc.vector.tensor_mul(vsel[:], val[:], msk[:])

        # ---- PE reduce: contract partitions within each group ------------
        # stream j = s*3200 + t*16 + q ; accumulate over t blocks of 8
        psum = ppool.tile([GROUPS, 512], dt.float32, tag="psum")
        v4 = vsel[:].rearrange("p (s t q) -> p s t q", s=SLOTS, t=T)
        nblk = T // 8  # 25 accumulation steps
        for r in range(nblk):
            nc.tensor.matmul(
                psum[:],
                ind_sb[:],
                v4[:, :, 8 * r : 8 * (r + 1), :],
                start=(r == 0),
                stop=(r == nblk - 1),
            )

        # ---- final 8-wide reduce + sigmoid -------------------------------
        # psum n = s*128 + ti*16 + q ; reduce ti, output col = s*16 + q
        red = pool.tile([GROUPS, COLS_PER_GROUP], dt.float32, tag="red")
        psum3 = psum[:].rearrange("g (s i q) -> g s q i", s=SLOTS, i=8)
        nc.vector.tensor_reduce(
            out=red[:], in_=psum3, axis=mybir.AxisListType.X, op=Alu.add
        )
        final = pool.tile([GROUPS, COLS_PER_GROUP], dt.float32, tag="final")
        nc.scalar.activation(
            out=final[:],
            in_=red[:],
            func=mybir.ActivationFunctionType.Sigmoid,
            bias=bias_sb[:, 0:1],
            scale=1.0,
        )
        nc.sync.dma_start(out_t[:], final[:])

    nc.finalize()
    return nc


def _get_program():
    if "prog" not in _prog_cache:
        _prog_cache["prog"] = _build_program()
    return _prog_cache["prog"]


def kernel(text, W, b):
    from concourse.bass_utils import run_bass_kernel_spmd

    text = np.asarray(text)
    W = np.asarray(W, dtype=np.float32).reshape(-1)
    b = np.asarray(b, dtype=np.float32).reshape(-1)
    x = text.astype(np.int32)  # [T, B]

    # host-side constant marshalling (input-independent transforms only)
    Wp = np.zeros(16 * CHUNK, np.float32)
    Wp[:V] = W
    Wp[1] = 0.0  # pad token never contributes
    table = np.ascontiguousarray(np.tile(Wp.reshape(16, CHUNK), (GROUPS, 1)))
    masktab = (
        (np.arange(T2N)[None, :] >> 9) == (np.arange(128)[:, None] % 16)
    ).astype(np.float32)
    ind = np.zeros((128, GROUPS), np.float32)
    ind[np.arange(128), np.arange(128) // 16] = 1.0
    bias = np.full((GROUPS, 1), b[0], np.float32)

    in_maps = []
    for d in range(NC_COUNT):
        tb = x[:, d * NCOL : (d + 1) * NCOL]  # [200, 512]
        tbr = tb.reshape(T, GROUPS, SLOTS, 16)  # [t, g, s, q]
        dev = np.ascontiguousarray(tbr.transpose(1, 3, 2, 0).reshape(128, SLOTS * T))
        in_maps.append(
            {
                "text_cols": dev,
                "table": table,
                "masktab": masktab,
                "ind": ind,
                "bias": bias,
            }
        )

    prog = _get_program()
    res = run_bass_kernel_spmd(prog, in_maps, core_ids=list(range(NC_COUNT)))

    out = np.empty((B,), np.float32)
    for d in range(NC_COUNT):
        out[d * NCOL : (d + 1) * NCOL] = res.results[d]["scores"].reshape(NCOL)
    return out.reshape(B, 1)


def benchmark(text, W, b, iters=20):
    """Estimate device execution time: device-resident inputs, repeated
    dispatch of the compiled 8-core program, min wall time per iteration."""
    import time

    import jax
    import numpy as np
    from jax.sharding import Mesh, PartitionSpec
    from jax.experimental.shard_map import shard_map
    from concourse import bass2jax
    import concourse.mybir as mybir

    prog = _get_program()
    # reuse kernel() marshalling
    text = np.asarray(text)
    W = np.asarray(W, dtype=np.float32).reshape(-1)
    b = np.asarray(b, dtype=np.float32).reshape(-1)
    x = text.astype(np.int32)
    Wp = np.zeros(16 * CHUNK, np.float32)
    Wp[:V] = W
    Wp[1] = 0.0
    table = np.ascontiguousarray(np.tile(Wp.reshape(16, CHUNK), (GROUPS, 1)))
    masktab = (
        (np.arange(T2N)[None, :] >> 9) == (np.arange(128)[:, None] % 16)
    ).astype(np.float32)
    ind = np.zeros((128, GROUPS), np.float32)
    ind[np.arange(128), np.arange(128) // 16] = 1.0
    bias = np.full((GROUPS, 1), b[0], np.float32)
    in_maps = []
    for d in range(NC_COUNT):
        tb = x[:, d * NCOL : (d + 1) * NCOL]
        tbr = tb.reshape(T, GROUPS, SLOTS, 16)
        dev = np.ascontiguousarray(tbr.transpose(1, 3, 2, 0).reshape(128, SLOTS * T))
        in_maps.append(
            {"text_cols": dev, "table": table, "masktab": masktab, "ind": ind, "bias": bias}
        )

    bass2jax.install_neuronx_cc_hook()
    nc = prog
    partition_name = nc.partition_id_tensor.name if nc.partition_id_tensor else None
    in_names, out_names, out_avals, zero_outs = [], [], [], []
    for alloc in nc.m.functions[0].allocations:
        if not isinstance(alloc, mybir.MemoryLocationSet):
            continue
        name = alloc.memorylocations[0].name
        if alloc.kind == "ExternalInput":
            if name != partition_name:
                in_names.append(name)
        elif alloc.kind == "ExternalOutput":
            out_names.append(name)
            shape = tuple(alloc.tensor_shape)
            dtype = mybir.dt.np(alloc.dtype)
            out_avals.append(jax.core.ShapedArray(shape, dtype))
            zero_outs.append(np.zeros(shape, dtype))
    n_params = len(in_names)
    n_outs = len(out_avals)
    all_names = in_names + out_names
    if partition_name is not None:
        all_names = all_names + [partition_name]

    def _body(*args):
        operands = list(args)
        if partition_name is not None:
            operands.append(bass2jax.partition_id_tensor())
        outs = bass2jax._bass_exec_p.bind(
            *operands,
            out_avals=tuple(out_avals),
            in_names=tuple(all_names),
            out_names=tuple(out_names),
            lowering_input_output_aliases=(),
            sim_require_finite=True,
            sim_require_nnan=True,
            nc=nc,
        )
        return tuple(outs)

    devices = jax.devices()[:NC_COUNT]
    mesh = Mesh(np.asarray(devices), ("core",))
    in_specs = (PartitionSpec("core"),) * (n_params + n_outs)
    out_specs = (PartitionSpec("core"),) * n_outs
    donate = tuple(range(n_params, n_params + n_outs))
    fn = jax.jit(
        shard_map(_body, mesh=mesh, in_specs=in_specs, out_specs=out_specs, check_rep=False),
        donate_argnums=donate,
        keep_unused=True,
    )
    concat_in = [
        np.concatenate([np.asarray(in_maps[c][nm]) for c in range(NC_COUNT)], axis=0)
        for nm in in_names
    ]
    sh = jax.sharding.NamedSharding(mesh, PartitionSpec("core"))
    dev_in = [jax.device_put(a, sh) for a in concat_in]

    def one_iter():
        zs = [np.zeros((NC_COUNT * z.shape[0], *z.shape[1:]), z.dtype) for z in zero_outs]
        outs = fn(*dev_in, *zs)
        jax.block_until_ready(outs)
        return outs

    one_iter()  # warmup / compile
    times = []
    for _ in range(iters):
        t0 = time.perf_counter()
        one_iter()
        times.append(time.perf_counter() - t0)
    tmin = min(times)
    tmed = sorted(times)[len(times) // 2]
    return tmin, tmed
